# revision 100
# baseline (speedup 1.0000x reference)
"""Trainium2 Bass kernel for causal multi-head attention.

Problem: B=4, T=2048, D=1024, H=16, HD=64, fp32, causal, scale=1/sqrt(D).

Sharding: 4-way batch x 2-way head-group over 8 cores. Core c=(b,g) computes
heads g*8..g*8+7 for batch b and returns the partial output projection
(contracted over its 512 context columns); the host sums the two partials
per batch element and adds bo.

Per-core dataflow (cost model charges a matmul ap_out_free * cycles_per_row
only -- no charge for contraction or output partitions; fp8 DoubleRow runs
at 0.5 cycles/row):
  - Q/K projections (f32r) write fp8e4 Q8/K8 scaled by 1/sqrt(sqrt(D)) each.
    Scores S^T[tk=128, tq] run as fp8e4 DoubleRow matmuls at 0.5 cyc/col --
    2x under bf16: lhsT = K8 block [64, 2, 128] whose j=1 k-tile is zeroed,
    rhs = Q8 [64, 2, N] whose j=1 plane is arbitrary finite data (the next
    ft plane; a zeroed dummy plane backs the last ft), so the second k-tile
    contributes exactly zero while the instruction still gets DoubleRow rate.
  - Causal mask adds -2048 on diagonal blocks via a DoubleRow fp8e5 matmul
    (lhsT = two half-identities [64,2,128], rhs = host-precomputed mask);
    -2048 = -64 after the folded 1/32 score scale, so exp underflows to 0.
  - exp on the scalar engine over block PAIRS ([128,2,512] psum spanning two
    banks) to amortize the fixed activation access cost; output pt is bf16.
  - PV runs TRANSPOSED: lhsT = pt chunk [tk=128, tq=128] (stationary),
    rhs = V tile [tk=128, 65] bf16 (moving, 65 cols = HD + ones column) so
    each accumulation step charges 65 cols instead of 512, and the softmax
    denominator lands as psum column 64 => per-partition normalize on DVE
    (reciprocal + tensor_scalar), no partition-broadcast DMA bounce.
  - ctx (bf16) is transposed back to [feat, tok] with PE transpose against a
    bf16 identity, then the output projection runs in bf16.
  - Projections / out-proj / transposes are emitted as filler thunks paced
    into the attention stream so the PE never starves while the scalar
    engine works through the exps (the attention inner loop is ACT-heavy);
    tj2+tj3 units are interleaved into one era since tj3 alone is exp-bound.
  - All DRAM traffic is bf16 (inputs/weights host-cast, output partials
    upcast on the host): the cost model serializes every DMA transfer, so
    bytes are the only lever there.
  - Odd heads' S operands are mirrored into partition-0-based tiles by
    sbuf->sbuf DMA: DoubleRow matmuls reading at base partition 64 crash
    the device in the full program (layout-sensitive; probes pass).
"""

import os
import numpy as np
from contextlib import ExitStack

import ml_dtypes
import concourse.bass as bass
import concourse.tile as tile
from concourse import bacc
from concourse import mybir
from concourse.bass_utils import run_bass_kernel_spmd

F32 = mybir.dt.float32
F32R = mybir.dt.float32r
BF16 = mybir.dt.bfloat16
F8E4 = mybir.dt.float8e4
F8E5 = mybir.dt.float8e5
AF = mybir.ActivationFunctionType
OP = mybir.AluOpType
DR = mybir.MatmulPerfMode.DoubleRow

MASKVAL = -2048.0  # -64 after the folded 1/32 score scale


def build_mha_core(T, D, F, DOUT, HD=64, TQ=512, num_devices=1):
    """Build the per-core Bass program.

    T: tokens, D: model dim, F: feature columns owned by this core,
    DOUT: output projection width, HD: head dim, TQ: tq tile width.
    """
    NH = F // HD        # local heads (8)
    DT = D // 128       # contraction tiles for projections (8)
    FT = F // 128       # feature 128-tiles (4)
    FTG = FT // 2       # q8/k8 feature tile groups (2)
    NTOK = T // 128     # token 128-tiles (16)
    NTQ = T // TQ       # tq tiles (4)
    NR = TQ // 128      # 128-blocks per tq tile (4)

    nc = bacc.Bacc(None, target_bir_lowering=False, debug=False, num_devices=num_devices)

    qT = nc.dram_tensor("qT", [D, T], BF16, kind="ExternalInput")
    kTd = nc.dram_tensor("kT", [D, T], BF16, kind="ExternalInput")
    vTd = nc.dram_tensor("vT", [D, T], BF16, kind="ExternalInput")
    Wq = nc.dram_tensor("Wq", [D, F], BF16, kind="ExternalInput")   # pre-scaled
    Wk = nc.dram_tensor("Wk", [D, F], BF16, kind="ExternalInput")   # pre-scaled
    Wv = nc.dram_tensor("Wv", [D, F], BF16, kind="ExternalInput")
    Wo = nc.dram_tensor("Wo", [F, DOUT], BF16, kind="ExternalInput")
    bq = nc.dram_tensor("bq", [128, FT], F32, kind="ExternalInput")  # pre-scaled
    bk = nc.dram_tensor("bk", [128, FT], F32, kind="ExternalInput")
    bv = nc.dram_tensor("bv", [1, F], F32, kind="ExternalInput")
    mneg = nc.dram_tensor("mneg", [64, 2, NR, TQ], F8E5, kind="ExternalInput")
    idHL = nc.dram_tensor("idHL", [64, 2, 128], F8E5, kind="ExternalInput")
    idT = nc.dram_tensor("idT", [128, 128], BF16, kind="ExternalInput")
    out = nc.dram_tensor("out", [T, DOUT], BF16, kind="ExternalOutput")

    with tile.TileContext(nc) as tc:
        with ExitStack() as ctx:
            persist = ctx.enter_context(tc.tile_pool(name="persist", bufs=1))
            Q8_sb = persist.tile([128, FT + 1, T], F8E4)
            K8_sb = persist.tile([128, FT, NTOK, 2, 128], F8E4)
            # Odd heads' operands live at partitions 64..127 of Q8/K8, but
            # DoubleRow matmuls reading at base partition 64 crash the device
            # in the full program (build-layout sensitive; standalone probes
            # pass). Mirror the upper halves into base-0 tiles via sbuf->sbuf
            # DMA and read S operands at base 0 for every head.
            Q8b = persist.tile([64, FT + 1, T], F8E4)
            K8b = persist.tile([64, FT, NTOK, 2, 128], F8E4)
            VA_sb = persist.tile([128, NTOK, NH, HD + 1], BF16)
            CTXN = persist.tile([128, NTOK, F], BF16)       # [tq-part, tok-blk, feat]
            CTXT = persist.tile([128, FT, T], BF16)         # [feat-part, ft, tok]
            Wq_sb = persist.tile([128, DT, F], BF16)
            Wk_sb = persist.tile([128, DT, F], BF16)
            Wv_sb = persist.tile([128, DT, F], BF16)
            Wo_sb = persist.tile([128, FT, DOUT], BF16)
            PART = persist.tile([128, 2 * NR, TQ], F32)  # tail outproj partials
            bq_sb = persist.tile([128, FT], F32)
            bk_sb = persist.tile([128, FT], F32)
            bv_sb = persist.tile([128, F], F32)
            mneg_sb = persist.tile([64, 2, NR, TQ], F8E5)
            idHL_sb = persist.tile([64, 2, 128], F8E5)
            idT_sb = persist.tile([128, 128], BF16)

            # PSUM: pS 2x[128,2,512]f32 (2 banks each) + pPV 2x[128,4,65]
            # (1 bank) + pmix 2x[128,512] (1 bank) = 8 banks exactly.
            pS = ctx.enter_context(tc.tile_pool(name="pS", bufs=2, space="PSUM"))
            pPV = ctx.enter_context(tc.tile_pool(name="pPV", bufs=2, space="PSUM"))
            pmix = ctx.enter_context(tc.tile_pool(name="pmix", bufs=2, space="PSUM"))
            ptp = ctx.enter_context(tc.tile_pool(name="ptp", bufs=4))
            xpool = ctx.enter_context(tc.tile_pool(name="xin", bufs=4))
            rpool = ctx.enter_context(tc.tile_pool(name="rcp", bufs=3))
            opool = ctx.enter_context(tc.tile_pool(name="osb", bufs=3))

            # Zero-plane memsets run on the (otherwise idle) Pool engine and
            # as 4-byte views: as plain DVE fp8 memsets they cost ~13us and
            # head-of-line-block the projection copybacks behind them.
            nc.vector.memset(VA_sb[:, :, :, HD:HD + 1], 1.0)
            nc.gpsimd.memset(K8_sb[:, :, :, 1, :].bitcast(F32), 0.0)
            nc.gpsimd.memset(Q8_sb[:, FT, :].bitcast(F32), 0.0)
            nc.gpsimd.memset(K8b[:, :, :, 1, :].bitcast(F32), 0.0)
            nc.gpsimd.memset(Q8b[:, FT, :].bitcast(F32), 0.0)

            def load_w(dst, wdram):
                wr = wdram[:].rearrange("(dt p) f -> p dt f", p=128)
                nc.sync.dma_start(dst[:], wr[:])

            # ---- filler thunk machinery ----
            filler = []  # (weight_us, thunk) pairs

            def drain_filler():
                while filler:
                    filler.pop(0)[1]()

            xstash = {}

            def prefetch_x(xdram, key, tj):
                def go():
                    t_ = xpool.tile([128, DT, TQ], BF16, tag="xin")
                    src = xdram[:, tj * TQ:(tj + 1) * TQ] \
                        .rearrange("(dt p) t -> p dt t", p=128)
                    nc.sync.dma_start(t_[:], src)
                    xstash[key] = t_
                return go

            def qk_proj_ft(which, tj, ft):
                """Project one 128-feature tile; write fp8 into Q8/K8."""
                Wsb, bsb = (Wq_sb, bq_sb) if which == "q" else (Wk_sb, bk_sb)
                key = (which, tj)

                def go():
                    xt = xstash[key]
                    ps = pmix.tile([128, TQ], F32, tag="pmix")
                    for dt in range(DT):
                        nc.tensor.matmul(
                            ps[:],
                            lhsT=Wsb[:, dt, ft * 128:(ft + 1) * 128],
                            rhs=xt[:, dt, :],
                            start=(dt == 0), stop=(dt == DT - 1))
                    if which == "q":
                        dst = Q8_sb[:, ft, tj * TQ:(tj + 1) * TQ]
                    else:
                        dst = K8_sb[:, ft, tj * NR:(tj + 1) * NR, 0, :]
                    nc.vector.tensor_scalar(
                        dst, ps[:], bsb[:, ft:ft + 1], None, OP.add)
                    # mirror the odd head's half to the base-0 tile
                    if which == "q":
                        nc.sync.dma_start(
                            Q8b[:, ft, tj * TQ:(tj + 1) * TQ],
                            Q8_sb[64:128, ft, tj * TQ:(tj + 1) * TQ])
                    else:
                        nc.sync.dma_start(
                            K8b[:, ft, tj * NR:(tj + 1) * NR, 0, :],
                            K8_sb[64:128, ft, tj * NR:(tj + 1) * NR, 0, :])
                return go

            def v_proj_c(key, tj, c):
                """Project one 128-token block of V (+bias) into VA bf16."""
                def go():
                    xt = xstash[key]
                    tt = tj * NR + c
                    ps = pmix.tile([128, TQ], F32, tag="pmix")
                    psv = ps[:, :F]
                    for dt in range(DT):
                        nc.tensor.matmul(
                            psv,
                            lhsT=xt[:, dt, c * 128:(c + 1) * 128],
                            rhs=Wv_sb[:, dt, :],
                            start=(dt == 0), stop=(dt == DT - 1))
                    nc.vector.tensor_tensor(
                        VA_sb[:, tt, :, 0:HD],
                        psv.rearrange("p (h d) -> p h d", h=NH),
                        bv_sb[:].rearrange("p (h d) -> p h d", h=NH),
                        OP.add)
                return go

            def proj_thunks(which, tj, skip_prefetch=False):
                """Thunk list for projecting input `which` at tile tj."""
                key = (which, tj)
                if which == "v":
                    th = [v_proj_c(key, tj, c) for c in range(NR)]
                else:
                    th = [qk_proj_ft(which, tj, ft) for ft in range(FT)]
                if not skip_prefetch:
                    xdram = {"q": qT, "k": kTd, "v": vTd}[which]
                    th = [prefetch_x(xdram, key, tj)] + th
                return th

            def transp_thunk(tt, fc):
                """Transpose ctx chunk [tq=128, feat=128] -> CTXT."""
                def go():
                    pt_ = pmix.tile([128, 128], BF16, tag="pmix")
                    nc.tensor.transpose(
                        pt_[:], CTXN[:, tt, fc * 128:(fc + 1) * 128], idT_sb[:])
                    nc.vector.tensor_copy(
                        CTXT[:, fc, tt * 128:(tt + 1) * 128], pt_[:])
                return go

            ostash = {}

            def outproj_part_thunk(tt, n):
                """ft 0..2 partial for a tail token block (heads 0..5 only)."""
                def go():
                    ps = pmix.tile([128, TQ], F32, tag="pmix")
                    for ft in range(FT - 1):
                        nc.tensor.matmul(
                            ps[:],
                            lhsT=CTXT[:, ft, tt * 128:(tt + 1) * 128],
                            rhs=Wo_sb[:, ft, n * TQ:(n + 1) * TQ],
                            start=(ft == 0), stop=(ft == FT - 2))
                    nc.vector.tensor_copy(PART[:, (tt - 12) * 2 + n, :], ps[:])
                return go

            def outproj_fin_thunk(tt, n):
                """last-ft matmul + partial add fused into the evacuation."""
                def go():
                    ps = pmix.tile([128, TQ], F32, tag="pmix")
                    nc.tensor.matmul(
                        ps[:],
                        lhsT=CTXT[:, FT - 1, tt * 128:(tt + 1) * 128],
                        rhs=Wo_sb[:, FT - 1, n * TQ:(n + 1) * TQ],
                        start=True, stop=True)
                    if n == 0:
                        ot_tile = opool.tile([128, DOUT], BF16, tag="ot")
                        ostash[tt] = ot_tile
                    ot = ostash[tt]
                    nc.vector.tensor_tensor(
                        ot[:, n * TQ:(n + 1) * TQ], ps[:],
                        PART[:, (tt - 12) * 2 + n, :], OP.add)
                    if n == DOUT // TQ - 1:
                        nc.sync.dma_start(
                            out[tt * 128:(tt + 1) * 128, :], ot[:])
                        del ostash[tt]
                return go

            def outproj_thunk(tt, n):
                def go():
                    ps = pmix.tile([128, TQ], F32, tag="pmix")
                    for ft in range(FT):
                        nc.tensor.matmul(
                            ps[:],
                            lhsT=CTXT[:, ft, tt * 128:(tt + 1) * 128],
                            rhs=Wo_sb[:, ft, n * TQ:(n + 1) * TQ],
                            start=(ft == 0), stop=(ft == FT - 1))
                    if n == 0:
                        ot_tile = opool.tile([128, DOUT], BF16, tag="ot")
                        ostash[tt] = ot_tile
                    ot = ostash[tt]
                    nc.vector.tensor_copy(ot[:, n * TQ:(n + 1) * TQ], ps[:])
                    if n == DOUT // TQ - 1:
                        nc.sync.dma_start(
                            out[tt * 128:(tt + 1) * 128, :], ot[:])
                        del ostash[tt]
                return go

            # ---- attention unit ----
            def emit_S_half(pS_t, half, h, tj, i):
                ft = h // 2
                Ksrc, Qsrc = (K8_sb, Q8_sb) if h % 2 == 0 else (K8b, Q8b)
                r = i - NR * tj
                c0 = 128 * r if r > 0 else 0
                # The exp reads the pair tile from the pair's base offset, so
                # the mask matmul must initialize (fully-masked) columns down
                # to the first half's offset on the steeper half.
                c0m = 128 * (r - half) if r - half > 0 else 0
                chunks = []
                n0 = c0
                while n0 < TQ:
                    chunks.append((n0, min(n0 + 256, TQ), "S"))
                    n0 = min(n0 + 256, TQ)
                if r >= 0:
                    # mask-only strip (no S written there) must be its own
                    # chunk: a matmul may not straddle the pending-zero edge
                    if c0m < c0:
                        chunks.append((c0m, c0, "M"))
                    n0 = c0
                    while n0 < TQ:
                        chunks.append((n0, min(n0 + 256, TQ), "M"))
                        n0 = min(n0 + 256, TQ)
                for idx, (a, b, kind) in enumerate(chunks):
                    last = idx == len(chunks) - 1
                    if kind == "S":
                        nc.tensor.matmul(
                            pS_t[:, half, a:b],
                            lhsT=Ksrc[0:64, ft, i, :, :],
                            rhs=Qsrc[0:64, ft:ft + 2, tj * TQ + a:tj * TQ + b],
                            start=(idx == 0), stop=last, perf_mode=DR)
                    else:
                        nc.tensor.matmul(
                            pS_t[:, half, a:b],
                            lhsT=idHL_sb[:],
                            rhs=mneg_sb[:, :, r, a:b],
                            start=False, stop=last, perf_mode=DR)

            def attention(h, tj, pace):
                nblk = NR * (tj + 1)
                pairs = nblk // 2

                def emit_pair(p):
                    t = pS.tile([128, 2, TQ], F32, tag="pS")
                    emit_S_half(t, 0, h, tj, 2 * p)
                    emit_S_half(t, 1, h, tj, 2 * p + 1)
                    return t

                pv_t = pPV.tile([128, NR, HD + 1], F32, tag="pPV")
                pS_cur = emit_pair(0)
                for p in range(pairs):
                    pS_next = emit_pair(p + 1) if p + 1 < pairs else None
                    r0 = 2 * p - NR * tj
                    c0p = 128 * r0 if r0 > 0 else 0
                    pt_t = ptp.tile([128, 2, TQ], BF16, tag="pt")
                    nc.scalar.activation(
                        pt_t[:, :, c0p:], pS_cur[:, :, c0p:], AF.Exp)
                    pace()
                    for half in (0, 1):
                        i = 2 * p + half
                        r = i - NR * tj
                        for q in range(max(r, 0), NR):
                            nc.tensor.matmul(
                                pv_t[:, q, :],
                                lhsT=pt_t[:, half, q * 128:(q + 1) * 128],
                                rhs=VA_sb[:, i, h, :],
                                start=(i == 0 and q == 0),
                                stop=(i == nblk - 1 and q == NR - 1))
                    pS_cur = pS_next
                # normalize: reciprocal of denominators, then scale ctx rows
                rcp = rpool.tile([128, NR], F32, tag="rcp")
                nc.vector.reciprocal(rcp[:], pv_t[:, :, HD])
                for r in range(NR):
                    nc.vector.tensor_scalar(
                        CTXN[:, tj * NR + r, h * HD:(h + 1) * HD],
                        pv_t[:, r, 0:HD],
                        rcp[:, r:r + 1], None, OP.mult)

            # ---- prologue: weights + tj=0 projections, DMA just-in-time.
            # The serial DMA stream is ordered so the first v-proj matmul's
            # exact operands (Wv dt 0-3 + xv dt 0-3, cols 0-255) land first;
            # everything else (second halves, bias) queues behind.
            wvr = Wv[:].rearrange("(dt p) f -> p dt f", p=128)
            qdt = DT // 4
            xv0 = xpool.tile([128, DT, TQ], BF16, tag="xin")
            xv0_src = vTd[:, 0:TQ].rearrange("(dt p) t -> p dt t", p=128)
            for i in range(4):
                a, b_ = i * qdt, (i + 1) * qdt
                nc.sync.dma_start(Wv_sb[:, a:b_, :], wvr[:, a:b_, :])
                nc.sync.dma_start(
                    xv0[:, a:b_, :TQ // 2], xv0_src[:, a:b_, :TQ // 2])
            nc.sync.dma_start(bv_sb[:], bv[:].to_broadcast([128, F]))
            xstash[("v", 0)] = xv0
            vth = proj_thunks("v", 0, True)
            vth[0]()
            vth[1]()
            nc.sync.dma_start(xv0[:, :, TQ // 2:], xv0_src[:, :, TQ // 2:])
            vth[2]()
            vth[3]()
            # q AND k loads issue before the q-proj thunks: xk0 must not sit
            # behind the q-mirror DMAs or the k-projection stalls ~3us
            load_w(Wq_sb, Wq)
            nc.sync.dma_start(bq_sb[:], bq[:])
            prefetch_x(qT, ("q", 0), 0)()
            load_w(Wk_sb, Wk)
            nc.sync.dma_start(bk_sb[:], bk[:])
            prefetch_x(kTd, ("k", 0), 0)()
            nc.sync.dma_start(mneg_sb[:], mneg[:])
            nc.sync.dma_start(idHL_sb[:], idHL[:])
            for th in proj_thunks("q", 0, True):
                th()
            for th in proj_thunks("k", 0, True):
                th()
            # era0's tj1 prefetches issue from the prologue for DMA lead
            prefetch_x(vTd, ("v", 1), 1)()
            prefetch_x(qT, ("q", 1), 1)()
            prefetch_x(kTd, ("k", 1), 1)()
            nc.sync.dma_start(idT_sb[:], idT[:])
            nc.sync.dma_start(
                Wo_sb[:], Wo[:].rearrange("(ft p) n -> p ft n", p=128))

            # ---- filler schedule per tj stretch ----
            # tj0 hosts proj(1); tj1 hosts proj(2); tj2 hosts qproj(3),
            # outproj(0), transp(0); tj3 hosts k/v proj(3) first (needed by
            # its own diagonal), then transp/outproj of tj 1-2. Remaining
            # tail after tj3: transp(3)+outproj(3).
            # ---- eras: tj0 | tj1 | tj2+tj3 interleaved ----
            # tj3 units are ACT(exp)-heavy vs their own PE work; mixing them
            # with tj2 units plus all deferrable PE filler keeps the tensor
            # engine dense while the scalar engine grinds through the exps.
            # `front` thunks are dependency-critical (this era's own K/V
            # projections, popped exactly 1/pair so each lands just before
            # its first consumer); `back` thunks are freely placeable filler
            # paced evenly over the era's remaining pairs.
            eras = [
                [(0, h) for h in range(NH)],
                [(1, h) for h in range(NH)],
                [(2, 0), (2, 1), (2, 2), (3, 0), (2, 3), (2, 4), (3, 1),
                 (2, 5), (2, 6), (3, 2), (2, 7), (3, 3), (3, 4), (3, 5),
                 (3, 6), (3, 7)],
            ]

            # thunk weights = approximate PE microseconds
            W_PROJ, W_OP, W_TR = 1.71, 0.85, 0.053

            def wrap(w_, ths):
                return [(w_, t) for t in ths]

            def era_sched(e):
                front, back = [], []
                if e == 0:
                    back += wrap(W_PROJ, proj_thunks("q", 1, True))
                elif e == 1:
                    # k(1)/v(1) self-feed: h0 needs K8 blocks 4,5 by its
                    # pair-2 S emission and VA 4..7 by pairs 2-3. front =
                    # (deadline_pair, thunk), popped just-in-time.
                    kf1 = [qk_proj_ft("k", 1, ft) for ft in range(FT)]
                    vc1 = [v_proj_c(("v", 1), 1, c) for c in range(NR)]
                    front += [(1, kf1[0]), (2, vc1[0]), (3, vc1[1]), (3, vc1[2]),
                              (4, vc1[3]), (7, kf1[1]), (15, kf1[2]), (23, kf1[3])]
                    back += wrap(0, [prefetch_x(qT, ("q", 2), 2)])
                    back += wrap(W_PROJ, proj_thunks("q", 2, True))
                    back += wrap(0, [prefetch_x(kTd, ("k", 2), 2),
                                     prefetch_x(vTd, ("v", 2), 2),
                                     prefetch_x(qT, ("q", 3), 3),
                                     prefetch_x(kTd, ("k", 3), 3),
                                     prefetch_x(vTd, ("v", 3), 3)])
                else:
                    kf2 = [qk_proj_ft("k", 2, ft) for ft in range(FT)]
                    vc2 = [v_proj_c(("v", 2), 2, c) for c in range(NR)]
                    kf3 = [qk_proj_ft("k", 3, ft) for ft in range(FT)]
                    vc3 = [v_proj_c(("v", 3), 3, c) for c in range(NR)]
                    qf3 = [qk_proj_ft("q", 3, ft) for ft in range(FT)]
                    front += [(1, kf2[0]), (2, vc2[0]), (3, vc2[1]), (4, vc2[2]),
                              (5, vc2[3]), (12, kf2[1]),
                              (13, qf3[0]), (14, qf3[1]), (15, qf3[2]), (16, qf3[3]),
                              (20, kf3[0]), (21, vc3[0]), (22, vc3[1]),
                              (23, vc3[2]), (24, vc3[3]),
                              (30, kf2[2]), (38, kf3[1]), (50, kf2[3]),
                              (76, kf3[2]), (92, kf3[3])]
                    back += wrap(W_TR, [transp_thunk(tt, fc)
                                        for tt in range(0, 8) for fc in range(FT)])
                    back += wrap(W_OP, [outproj_thunk(tt, n)
                                        for tt in range(0, 8) for n in range(DOUT // TQ)])
                return front, back

            # per-pair PE deficit budget (us): era0/1 sized to drain their
            # filler within the era; era2 sized to the exp-vs-attention gap
            DPP = [0.60, 0.65, 0.57]

            TRUNC = int(os.environ.get("MHA_TRUNC", "99"))  # debug bisect knob
            NUNITS = int(os.environ.get("MHA_UNITS", "999"))
            for e, units in enumerate(eras):
                if e >= TRUNC:
                    break
                hoff = int(os.environ.get("MHA_HOFF", "0"))
                units = units[hoff:hoff + NUNITS]
                front, b = era_sched(e)
                filler.extend(b)
                state = [0.0, 0.0, 0]  # budget, spent, pairs

                def pace(state=state, dpp=DPP[e], front=front):
                    if os.environ.get("MHA_NOFILL"):
                        return
                    state[0] += dpp
                    state[2] += 1
                    while front and front[0][0] <= state[2] + 3:
                        state[1] += W_PROJ
                        front.pop(0)[1]()
                    while filler and state[1] + filler[0][0] * 0.5 <= state[0]:
                        w_, fn = filler.pop(0)
                        state[1] += w_
                        fn()

                for tj, h in units:
                    attention(h, tj, pace)
                    if (tj, h) == (2, 7):
                        filler.extend(wrap(W_TR, [transp_thunk(tt, fc)
                                      for tt in range(8, 12) for fc in range(FT)]))
                        filler.extend(wrap(W_OP, [outproj_thunk(tt, n)
                                      for tt in range(8, 12) for n in range(DOUT // TQ)]))
                    elif tj == 3 and h in (1, 3, 5):
                        filler.extend(wrap(W_TR, [
                            transp_thunk(tt, h // 2) for tt in range(12, 16)]))
                        if h == 5:
                            filler.extend(wrap(0.64, [
                                outproj_part_thunk(tt, n) for tt in range(12, 16)
                                for n in range(DOUT // TQ)]))
                while front:
                    front.pop(0)[1]()
                drain_filler()

            if TRUNC >= 99:
                for tt in range(12, 16):
                    transp_thunk(tt, FT - 1)()
                    for n in range(DOUT // TQ):
                        outproj_fin_thunk(tt, n)()

    nc.compile()
    return nc


def make_mask(TQ=512, NR=4):
    """mneg[p, jj, r, f] = MASKVAL where f < tk_local (tk_local = 64*jj + p
    + 128*r), i.e. query strictly before key inside diagonal block r."""
    p = np.arange(64)[:, None, None, None]
    jj = np.arange(2)[None, :, None, None]
    r = np.arange(NR)[None, None, :, None]
    f = np.arange(TQ)[None, None, None, :]
    m = np.where(f < 64 * jj + p + 128 * r, np.float32(MASKVAL), np.float32(0.0))
    return m.astype(ml_dtypes.float8_e5m2)


def make_idHL():
    """idHL[p, jj, m] = 1 where m == 64*jj + p (two stacked half-identities)."""
    m = np.zeros((64, 2, 128), np.float32)
    for jj in range(2):
        m[np.arange(64), jj, 64 * jj + np.arange(64)] = 1.0
    return m.astype(ml_dtypes.float8_e5m2)


def make_core_inputs(q_b, k_b, v_b, Wq, bq, Wk, bk, Wv, bv, Wo, fsl, TQ=512):
    """Build the in_map for one core. fsl = feature slice for this core's heads."""
    F = fsl.stop - fsl.start
    FT = F // 128
    NR = TQ // 128
    D = Wq.shape[0]
    s8 = np.float32(1.0 / np.sqrt(np.sqrt(np.float32(D))))  # 1/sqrt(32)

    bf = ml_dtypes.bfloat16
    return {
        "qT": np.ascontiguousarray(q_b.T).astype(bf),
        "kT": np.ascontiguousarray(k_b.T).astype(bf),
        "vT": np.ascontiguousarray(v_b.T).astype(bf),
        "Wq": np.ascontiguousarray(np.asarray(Wq[:, fsl]) * s8).astype(bf),
        "Wk": np.ascontiguousarray(np.asarray(Wk[:, fsl]) * s8).astype(bf),
        "Wv": np.ascontiguousarray(Wv[:, fsl]).astype(bf),
        "Wo": np.ascontiguousarray(Wo[fsl, :]).astype(bf),
        "bq": np.ascontiguousarray((np.asarray(bq[fsl]) * s8).reshape(FT, 128).T),
        "bk": np.ascontiguousarray((np.asarray(bk[fsl]) * s8).reshape(FT, 128).T),
        "bv": np.ascontiguousarray(bv[fsl].reshape(1, F)),
        "mneg": make_mask(TQ, NR),
        "idHL": make_idHL(),
        "idT": np.eye(128, dtype=np.float32).astype(ml_dtypes.bfloat16),
    }


_CACHE = {}


def kernel(q, k, v, Wq, bq, Wk, bk, Wv, bv, Wo, bo, _trace=False):
    B, T, D = q.shape
    H, HD = 16, 64
    n_cores = 8
    gpb = n_cores // B            # head-groups per batch element (2)
    F = D // gpb                  # feature columns per core (512)

    key = (T, D, F)
    if key not in _CACHE:
        _CACHE[key] = build_mha_core(T=T, D=D, F=F, DOUT=D, HD=HD, TQ=512,
                                     num_devices=n_cores)
    nc = _CACHE[key]

    q = np.asarray(q, np.float32)
    k = np.asarray(k, np.float32)
    v = np.asarray(v, np.float32)
    in_maps = []
    for c in range(n_cores):
        b, g = c // gpb, c % gpb
        fsl = slice(g * F, (g + 1) * F)
        in_maps.append(make_core_inputs(
            q[b], k[b], v[b], Wq, bq, Wk, bk, Wv, bv, Wo, fsl))

    res = run_bass_kernel_spmd(nc, in_maps, list(range(n_cores)), trace=_trace)
    out = np.zeros((B, T, D), np.float32)
    for c in range(n_cores):
        out[c // gpb] += np.asarray(res.results[c]["out"], np.float32)
    out += np.asarray(bo, np.float32)
    if _trace:
        kernel.last_exec_time_ns = res.exec_time_ns
    return out


# revision 101
# speedup vs baseline: 1.0132x; 1.0132x over previous
"""Trainium2 Bass kernel for causal multi-head attention.

Problem: B=4, T=2048, D=1024, H=16, HD=64, fp32, causal, scale=1/sqrt(D).

Sharding: 4-way batch x 2-way head-group over 8 cores. Core c=(b,g) computes
heads g*8..g*8+7 for batch b and returns the partial output projection
(contracted over its 512 context columns); the host sums the two partials
per batch element and adds bo.

Per-core dataflow (cost model charges a matmul ap_out_free * cycles_per_row
only -- no charge for contraction or output partitions; fp8 DoubleRow runs
at 0.5 cycles/row):
  - Q/K projections (f32r) write fp8e4 Q8/K8 scaled by 1/sqrt(sqrt(D)) each.
    Scores S^T[tk=128, tq] run as fp8e4 DoubleRow matmuls at 0.5 cyc/col --
    2x under bf16: lhsT = K8 block [64, 2, 128] whose j=1 k-tile is zeroed,
    rhs = Q8 [64, 2, N] whose j=1 plane is arbitrary finite data (the next
    ft plane; a zeroed dummy plane backs the last ft), so the second k-tile
    contributes exactly zero while the instruction still gets DoubleRow rate.
  - Causal mask adds -2048 on diagonal blocks via a DoubleRow fp8e5 matmul
    (lhsT = two half-identities [64,2,128], rhs = host-precomputed mask);
    -2048 = -64 after the folded 1/32 score scale, so exp underflows to 0.
  - exp on the scalar engine over block PAIRS ([128,2,512] psum spanning two
    banks) to amortize the fixed activation access cost; output pt is bf16.
  - PV runs TRANSPOSED: lhsT = pt chunk [tk=128, tq=128] (stationary),
    rhs = V tile [tk=128, 65] bf16 (moving, 65 cols = HD + ones column) so
    each accumulation step charges 65 cols instead of 512, and the softmax
    denominator lands as psum column 64 => per-partition normalize on DVE
    (reciprocal + tensor_scalar), no partition-broadcast DMA bounce.
  - ctx (bf16) is transposed back to [feat, tok] with PE transpose against a
    bf16 identity, then the output projection runs in bf16.
  - Projections / out-proj / transposes are emitted as filler thunks paced
    into the attention stream so the PE never starves while the scalar
    engine works through the exps (the attention inner loop is ACT-heavy);
    tj2+tj3 units are interleaved into one era since tj3 alone is exp-bound.
  - All DRAM traffic is bf16 (inputs/weights host-cast, output partials
    upcast on the host): the cost model serializes every DMA transfer, so
    bytes are the only lever there.
  - Odd heads' S operands are mirrored into partition-0-based tiles by
    sbuf->sbuf DMA: DoubleRow matmuls reading at base partition 64 crash
    the device in the full program (layout-sensitive; probes pass).
"""

import os
import numpy as np
from contextlib import ExitStack

import ml_dtypes
import concourse.bass as bass
import concourse.tile as tile
from concourse import bacc
from concourse import mybir
from concourse.bass_utils import run_bass_kernel_spmd

F32 = mybir.dt.float32
F32R = mybir.dt.float32r
BF16 = mybir.dt.bfloat16
F8E4 = mybir.dt.float8e4
F8E5 = mybir.dt.float8e5
AF = mybir.ActivationFunctionType
OP = mybir.AluOpType
DR = mybir.MatmulPerfMode.DoubleRow

MASKVAL = -2048.0  # -64 after the folded 1/32 score scale


def build_mha_core(T, D, F, DOUT, HD=64, TQ=512, num_devices=1):
    """Build the per-core Bass program.

    T: tokens, D: model dim, F: feature columns owned by this core,
    DOUT: output projection width, HD: head dim, TQ: tq tile width.
    """
    NH = F // HD        # local heads (8)
    DT = D // 128       # contraction tiles for projections (8)
    FT = F // 128       # feature 128-tiles (4)
    FTG = FT // 2       # q8/k8 feature tile groups (2)
    NTOK = T // 128     # token 128-tiles (16)
    NTQ = T // TQ       # tq tiles (4)
    NR = TQ // 128      # 128-blocks per tq tile (4)

    nc = bacc.Bacc(None, target_bir_lowering=False, debug=False, num_devices=num_devices)

    qT = nc.dram_tensor("qT", [D, T], BF16, kind="ExternalInput")
    kTd = nc.dram_tensor("kT", [D, T], BF16, kind="ExternalInput")
    vTd = nc.dram_tensor("vT", [D, T], BF16, kind="ExternalInput")
    Wq = nc.dram_tensor("Wq", [D, F], BF16, kind="ExternalInput")   # pre-scaled
    Wk = nc.dram_tensor("Wk", [D, F], BF16, kind="ExternalInput")   # pre-scaled
    Wv = nc.dram_tensor("Wv", [D, F], BF16, kind="ExternalInput")
    Wo = nc.dram_tensor("Wo", [F, DOUT], BF16, kind="ExternalInput")
    bq = nc.dram_tensor("bq", [128, FT], F32, kind="ExternalInput")  # pre-scaled
    bk = nc.dram_tensor("bk", [128, FT], F32, kind="ExternalInput")
    bv = nc.dram_tensor("bv", [1, F], F32, kind="ExternalInput")
    mneg = nc.dram_tensor("mneg", [64, 2, NR, TQ], F8E5, kind="ExternalInput")
    idHL = nc.dram_tensor("idHL", [64, 2, 128], F8E5, kind="ExternalInput")
    idT = nc.dram_tensor("idT", [128, 128], BF16, kind="ExternalInput")
    out = nc.dram_tensor("out", [T, DOUT], BF16, kind="ExternalOutput")

    with tile.TileContext(nc) as tc:
        with ExitStack() as ctx:
            persist = ctx.enter_context(tc.tile_pool(name="persist", bufs=1))
            Q8_sb = persist.tile([128, FT + 1, T], F8E4)
            K8_sb = persist.tile([128, FT, NTOK, 2, 128], F8E4)
            # Odd heads' operands live at partitions 64..127 of Q8/K8, but
            # DoubleRow matmuls reading at base partition 64 crash the device
            # in the full program (build-layout sensitive; standalone probes
            # pass). Mirror the upper halves into base-0 tiles via sbuf->sbuf
            # DMA and read S operands at base 0 for every head.
            Q8b = persist.tile([64, FT + 1, T], F8E4)
            K8b = persist.tile([64, FT, NTOK, 2, 128], F8E4)
            VA_sb = persist.tile([128, NTOK, NH, HD + 1], BF16)
            CTXN = persist.tile([128, NTOK, F], BF16)       # [tq-part, tok-blk, feat]
            CTXT = persist.tile([128, FT, T], BF16)         # [feat-part, ft, tok]
            Wq_sb = persist.tile([128, DT, F], BF16)
            Wk_sb = persist.tile([128, DT, F], BF16)
            Wv_sb = persist.tile([128, DT, F], BF16)
            Wo_sb = persist.tile([128, FT, DOUT], BF16)
            PART = persist.tile([128, 2 * NR, TQ], F32)  # tail outproj partials
            bq_sb = persist.tile([128, FT], F32)
            bk_sb = persist.tile([128, FT], F32)
            bv_sb = persist.tile([128, F], F32)
            mneg_sb = persist.tile([64, 2, NR, TQ], F8E5)
            idHL_sb = persist.tile([64, 2, 128], F8E5)
            idT_sb = persist.tile([128, 128], BF16)

            # PSUM: pS 2x[128,2,512]f32 (2 banks each) + pPV 2x[128,4,65]
            # (1 bank) + pmix 2x[128,512] (1 bank) = 8 banks exactly.
            pS = ctx.enter_context(tc.tile_pool(name="pS", bufs=2, space="PSUM"))
            pPV = ctx.enter_context(tc.tile_pool(name="pPV", bufs=2, space="PSUM"))
            pmix = ctx.enter_context(tc.tile_pool(name="pmix", bufs=2, space="PSUM"))
            ptp = ctx.enter_context(tc.tile_pool(name="ptp", bufs=4))
            xpool = ctx.enter_context(tc.tile_pool(name="xin", bufs=4))
            rpool = ctx.enter_context(tc.tile_pool(name="rcp", bufs=3))
            opool = ctx.enter_context(tc.tile_pool(name="osb", bufs=3))

            # Zero-plane memsets run on the (otherwise idle) Pool engine and
            # as 4-byte views: as plain DVE fp8 memsets they cost ~13us and
            # head-of-line-block the projection copybacks behind them.
            nc.vector.memset(VA_sb[:, :, :, HD:HD + 1], 1.0)
            nc.gpsimd.memset(K8_sb[:, :, :, 1, :].bitcast(F32), 0.0)
            nc.gpsimd.memset(Q8_sb[:, FT, :].bitcast(F32), 0.0)
            nc.gpsimd.memset(K8b[:, :, :, 1, :].bitcast(F32), 0.0)
            nc.gpsimd.memset(Q8b[:, FT, :].bitcast(F32), 0.0)

            def load_w(dst, wdram):
                wr = wdram[:].rearrange("(dt p) f -> p dt f", p=128)
                nc.sync.dma_start(dst[:], wr[:])

            # ---- filler thunk machinery ----
            filler = []  # (weight_us, thunk) pairs

            def drain_filler():
                while filler:
                    filler.pop(0)[1]()

            xstash = {}

            def prefetch_x(xdram, key, tj):
                def go():
                    t_ = xpool.tile([128, DT, TQ], BF16, tag="xin")
                    src = xdram[:, tj * TQ:(tj + 1) * TQ] \
                        .rearrange("(dt p) t -> p dt t", p=128)
                    nc.sync.dma_start(t_[:], src)
                    xstash[key] = t_
                return go

            def qk_proj_ft(which, tj, ft):
                """Project one 128-feature tile; write fp8 into Q8/K8."""
                Wsb, bsb = (Wq_sb, bq_sb) if which == "q" else (Wk_sb, bk_sb)
                key = (which, tj)

                def go():
                    xt = xstash[key]
                    ps = pmix.tile([128, TQ], F32, tag="pmix")
                    for dt in range(DT):
                        nc.tensor.matmul(
                            ps[:],
                            lhsT=Wsb[:, dt, ft * 128:(ft + 1) * 128],
                            rhs=xt[:, dt, :],
                            start=(dt == 0), stop=(dt == DT - 1))
                    if which == "q":
                        dst = Q8_sb[:, ft, tj * TQ:(tj + 1) * TQ]
                    else:
                        dst = K8_sb[:, ft, tj * NR:(tj + 1) * NR, 0, :]
                    nc.vector.tensor_scalar(
                        dst, ps[:], bsb[:, ft:ft + 1], None, OP.add)
                    # mirror the odd head's half to the base-0 tile
                    if which == "q":
                        nc.sync.dma_start(
                            Q8b[:, ft, tj * TQ:(tj + 1) * TQ],
                            Q8_sb[64:128, ft, tj * TQ:(tj + 1) * TQ])
                    else:
                        nc.sync.dma_start(
                            K8b[:, ft, tj * NR:(tj + 1) * NR, 0, :],
                            K8_sb[64:128, ft, tj * NR:(tj + 1) * NR, 0, :])
                return go

            def v_proj_c(key, tj, c):
                """Project one 128-token block of V (+bias) into VA bf16."""
                def go():
                    xt = xstash[key]
                    tt = tj * NR + c
                    ps = pmix.tile([128, TQ], F32, tag="pmix")
                    psv = ps[:, :F]
                    for dt in range(DT):
                        nc.tensor.matmul(
                            psv,
                            lhsT=xt[:, dt, c * 128:(c + 1) * 128],
                            rhs=Wv_sb[:, dt, :],
                            start=(dt == 0), stop=(dt == DT - 1))
                    nc.vector.tensor_tensor(
                        VA_sb[:, tt, :, 0:HD],
                        psv.rearrange("p (h d) -> p h d", h=NH),
                        bv_sb[:].rearrange("p (h d) -> p h d", h=NH),
                        OP.add)
                return go

            def proj_thunks(which, tj, skip_prefetch=False):
                """Thunk list for projecting input `which` at tile tj."""
                key = (which, tj)
                if which == "v":
                    th = [v_proj_c(key, tj, c) for c in range(NR)]
                else:
                    th = [qk_proj_ft(which, tj, ft) for ft in range(FT)]
                if not skip_prefetch:
                    xdram = {"q": qT, "k": kTd, "v": vTd}[which]
                    th = [prefetch_x(xdram, key, tj)] + th
                return th

            def transp_thunk(tt, fc):
                """Transpose ctx chunk [tq=128, feat=128] -> CTXT."""
                def go():
                    pt_ = pmix.tile([128, 128], BF16, tag="pmix")
                    nc.tensor.transpose(
                        pt_[:], CTXN[:, tt, fc * 128:(fc + 1) * 128], idT_sb[:])
                    nc.vector.tensor_copy(
                        CTXT[:, fc, tt * 128:(tt + 1) * 128], pt_[:])
                return go

            ostash = {}

            def outproj_part_thunk(tt, n):
                """ft 0..2 partial for a tail token block (heads 0..5 only)."""
                def go():
                    ps = pmix.tile([128, TQ], F32, tag="pmix")
                    for ft in range(FT - 1):
                        nc.tensor.matmul(
                            ps[:],
                            lhsT=CTXT[:, ft, tt * 128:(tt + 1) * 128],
                            rhs=Wo_sb[:, ft, n * TQ:(n + 1) * TQ],
                            start=(ft == 0), stop=(ft == FT - 2))
                    nc.vector.tensor_copy(PART[:, (tt - 12) * 2 + n, :], ps[:])
                return go

            def outproj_fin_thunk(tt, n):
                """last-ft matmul + partial add fused into the evacuation."""
                def go():
                    ps = pmix.tile([128, TQ], F32, tag="pmix")
                    nc.tensor.matmul(
                        ps[:],
                        lhsT=CTXT[:, FT - 1, tt * 128:(tt + 1) * 128],
                        rhs=Wo_sb[:, FT - 1, n * TQ:(n + 1) * TQ],
                        start=True, stop=True)
                    if n == 0:
                        ot_tile = opool.tile([128, DOUT], BF16, tag="ot")
                        ostash[tt] = ot_tile
                    ot = ostash[tt]
                    nc.vector.tensor_tensor(
                        ot[:, n * TQ:(n + 1) * TQ], ps[:],
                        PART[:, (tt - 12) * 2 + n, :], OP.add)
                    if n == DOUT // TQ - 1:
                        nc.sync.dma_start(
                            out[tt * 128:(tt + 1) * 128, :], ot[:])
                        del ostash[tt]
                return go

            def outproj_thunk(tt, n):
                def go():
                    ps = pmix.tile([128, TQ], F32, tag="pmix")
                    for ft in range(FT):
                        nc.tensor.matmul(
                            ps[:],
                            lhsT=CTXT[:, ft, tt * 128:(tt + 1) * 128],
                            rhs=Wo_sb[:, ft, n * TQ:(n + 1) * TQ],
                            start=(ft == 0), stop=(ft == FT - 1))
                    if n == 0:
                        ot_tile = opool.tile([128, DOUT], BF16, tag="ot")
                        ostash[tt] = ot_tile
                    ot = ostash[tt]
                    nc.vector.tensor_copy(ot[:, n * TQ:(n + 1) * TQ], ps[:])
                    if n == DOUT // TQ - 1:
                        nc.sync.dma_start(
                            out[tt * 128:(tt + 1) * 128, :], ot[:])
                        del ostash[tt]
                return go

            # ---- attention unit ----
            def emit_S_half(pS_t, half, h, tj, i):
                ft = h // 2
                Ksrc, Qsrc = (K8_sb, Q8_sb) if h % 2 == 0 else (K8b, Q8b)
                r = i - NR * tj
                c0 = 128 * r if r > 0 else 0
                # The exp reads the pair tile from the pair's base offset, so
                # the mask matmul must initialize (fully-masked) columns down
                # to the first half's offset on the steeper half.
                c0m = 128 * (r - half) if r - half > 0 else 0
                chunks = []
                n0 = c0
                while n0 < TQ:
                    chunks.append((n0, min(n0 + 256, TQ), "S"))
                    n0 = min(n0 + 256, TQ)
                if r >= 0:
                    # mask-only strip (no S written there) must be its own
                    # chunk: a matmul may not straddle the pending-zero edge
                    if c0m < c0:
                        chunks.append((c0m, c0, "M"))
                    n0 = c0
                    while n0 < TQ:
                        chunks.append((n0, min(n0 + 256, TQ), "M"))
                        n0 = min(n0 + 256, TQ)
                for idx, (a, b, kind) in enumerate(chunks):
                    last = idx == len(chunks) - 1
                    if kind == "S":
                        nc.tensor.matmul(
                            pS_t[:, half, a:b],
                            lhsT=Ksrc[0:64, ft, i, :, :],
                            rhs=Qsrc[0:64, ft:ft + 2, tj * TQ + a:tj * TQ + b],
                            start=(idx == 0), stop=last, perf_mode=DR)
                    else:
                        nc.tensor.matmul(
                            pS_t[:, half, a:b],
                            lhsT=idHL_sb[:],
                            rhs=mneg_sb[:, :, r, a:b],
                            start=False, stop=last, perf_mode=DR)

            def attention(h, tj, pace):
                nblk = NR * (tj + 1)
                pairs = nblk // 2

                def emit_pair(p):
                    t = pS.tile([128, 2, TQ], F32, tag="pS")
                    emit_S_half(t, 0, h, tj, 2 * p)
                    emit_S_half(t, 1, h, tj, 2 * p + 1)
                    return t

                pv_t = pPV.tile([128, NR, HD + 1], F32, tag="pPV")
                pS_cur = emit_pair(0)
                for p in range(pairs):
                    pS_next = emit_pair(p + 1) if p + 1 < pairs else None
                    r0 = 2 * p - NR * tj
                    c0p = 128 * r0 if r0 > 0 else 0
                    pt_t = ptp.tile([128, 2, TQ], BF16, tag="pt")
                    nc.scalar.activation(
                        pt_t[:, :, c0p:], pS_cur[:, :, c0p:], AF.Exp)
                    pace()
                    for half in (0, 1):
                        i = 2 * p + half
                        r = i - NR * tj
                        for q in range(max(r, 0), NR):
                            nc.tensor.matmul(
                                pv_t[:, q, :],
                                lhsT=pt_t[:, half, q * 128:(q + 1) * 128],
                                rhs=VA_sb[:, i, h, :],
                                start=(i == 0 and q == 0),
                                stop=(i == nblk - 1 and q == NR - 1))
                    pS_cur = pS_next
                # normalize: reciprocal of denominators, then scale ctx rows
                rcp = rpool.tile([128, NR], F32, tag="rcp")
                nc.vector.reciprocal(rcp[:], pv_t[:, :, HD])
                for r in range(NR):
                    nc.vector.tensor_scalar(
                        CTXN[:, tj * NR + r, h * HD:(h + 1) * HD],
                        pv_t[:, r, 0:HD],
                        rcp[:, r:r + 1], None, OP.mult)

            # ---- prologue: weights + tj=0 projections, DMA just-in-time.
            # The serial DMA stream is ordered so the first v-proj matmul's
            # exact operands (Wv dt 0-3 + xv dt 0-3, cols 0-255) land first;
            # everything else (second halves, bias) queues behind.
            wvr = Wv[:].rearrange("(dt p) f -> p dt f", p=128)
            qdt = DT // 4
            xv0 = xpool.tile([128, DT, TQ], BF16, tag="xin")
            xv0_src = vTd[:, 0:TQ].rearrange("(dt p) t -> p dt t", p=128)
            for i in range(4):
                a, b_ = i * qdt, (i + 1) * qdt
                nc.sync.dma_start(Wv_sb[:, a:b_, :], wvr[:, a:b_, :])
                nc.sync.dma_start(
                    xv0[:, a:b_, :TQ // 2], xv0_src[:, a:b_, :TQ // 2])
            nc.sync.dma_start(bv_sb[:], bv[:].to_broadcast([128, F]))
            xstash[("v", 0)] = xv0
            vth = proj_thunks("v", 0, True)
            vth[0]()
            vth[1]()
            nc.sync.dma_start(xv0[:, :, TQ // 2:], xv0_src[:, :, TQ // 2:])
            vth[2]()
            vth[3]()
            # q AND k loads issue before the q-proj thunks: xk0 must not sit
            # behind the q-mirror DMAs or the k-projection stalls ~3us
            load_w(Wq_sb, Wq)
            nc.sync.dma_start(bq_sb[:], bq[:])
            prefetch_x(qT, ("q", 0), 0)()
            load_w(Wk_sb, Wk)
            nc.sync.dma_start(bk_sb[:], bk[:])
            prefetch_x(kTd, ("k", 0), 0)()
            nc.sync.dma_start(mneg_sb[:], mneg[:])
            nc.sync.dma_start(idHL_sb[:], idHL[:])
            for th in proj_thunks("q", 0, True):
                th()
            for th in proj_thunks("k", 0, True):
                th()
            # era0's tj1 prefetches issue from the prologue for DMA lead.
            # q1 first: its proj thunks are era0 filler (consumed ~30us)
            # while v1/k1 feed era1's front (~47us).
            prefetch_x(qT, ("q", 1), 1)()
            prefetch_x(vTd, ("v", 1), 1)()
            prefetch_x(kTd, ("k", 1), 1)()
            nc.sync.dma_start(idT_sb[:], idT[:])
            nc.sync.dma_start(
                Wo_sb[:], Wo[:].rearrange("(ft p) n -> p ft n", p=128))

            # ---- filler schedule per tj stretch ----
            # tj0 hosts proj(1); tj1 hosts proj(2); tj2 hosts qproj(3),
            # outproj(0), transp(0); tj3 hosts k/v proj(3) first (needed by
            # its own diagonal), then transp/outproj of tj 1-2. Remaining
            # tail after tj3: transp(3)+outproj(3).
            # ---- eras: tj0 | tj1 | tj2+tj3 interleaved ----
            # tj3 units are ACT(exp)-heavy vs their own PE work; mixing them
            # with tj2 units plus all deferrable PE filler keeps the tensor
            # engine dense while the scalar engine grinds through the exps.
            # `front` thunks are dependency-critical (this era's own K/V
            # projections, popped exactly 1/pair so each lands just before
            # its first consumer); `back` thunks are freely placeable filler
            # paced evenly over the era's remaining pairs.
            eras = [
                [(0, h) for h in range(NH)],
                [(1, h) for h in range(NH)],
                [(2, 0), (2, 1), (2, 2), (3, 0), (2, 3), (2, 4), (3, 1),
                 (2, 5), (2, 6), (3, 2), (2, 7), (3, 3), (3, 4), (3, 5),
                 (3, 6), (3, 7)],
            ]

            # thunk weights = approximate PE microseconds
            W_PROJ, W_OP, W_TR = 1.71, 0.85, 0.053

            def wrap(w_, ths):
                return [(w_, t) for t in ths]

            def era_sched(e):
                front, back = [], []
                if e == 0:
                    back += wrap(W_PROJ, proj_thunks("q", 1, True))
                elif e == 1:
                    # k(1)/v(1) self-feed: h0 needs K8 blocks 4,5 by its
                    # pair-2 S emission and VA 4..7 by pairs 2-3. front =
                    # (deadline_pair, thunk), popped just-in-time.
                    kf1 = [qk_proj_ft("k", 1, ft) for ft in range(FT)]
                    vc1 = [v_proj_c(("v", 1), 1, c) for c in range(NR)]
                    front += [(1, kf1[0]), (2, vc1[0]), (3, vc1[1]), (3, vc1[2]),
                              (4, vc1[3]), (7, kf1[1]), (15, kf1[2]), (23, kf1[3])]
                    back += wrap(0, [prefetch_x(qT, ("q", 2), 2)])
                    back += wrap(W_PROJ, proj_thunks("q", 2, True))
                    back += wrap(0, [prefetch_x(kTd, ("k", 2), 2),
                                     prefetch_x(vTd, ("v", 2), 2),
                                     prefetch_x(qT, ("q", 3), 3),
                                     prefetch_x(kTd, ("k", 3), 3),
                                     prefetch_x(vTd, ("v", 3), 3)])
                else:
                    kf2 = [qk_proj_ft("k", 2, ft) for ft in range(FT)]
                    vc2 = [v_proj_c(("v", 2), 2, c) for c in range(NR)]
                    kf3 = [qk_proj_ft("k", 3, ft) for ft in range(FT)]
                    vc3 = [v_proj_c(("v", 3), 3, c) for c in range(NR)]
                    qf3 = [qk_proj_ft("q", 3, ft) for ft in range(FT)]
                    front += [(1, kf2[0]), (2, vc2[0]), (3, vc2[1]), (4, vc2[2]),
                              (5, vc2[3]), (12, kf2[1]),
                              (13, qf3[0]), (14, qf3[1]), (15, qf3[2]), (16, qf3[3]),
                              (20, kf3[0]), (21, vc3[0]), (22, vc3[1]),
                              (23, vc3[2]), (24, vc3[3]),
                              (30, kf2[2]), (38, kf3[1]), (50, kf2[3]),
                              (76, kf3[2]), (92, kf3[3])]
                    back += wrap(W_TR, [transp_thunk(tt, fc)
                                        for tt in range(0, 8) for fc in range(FT)])
                    back += wrap(W_OP, [outproj_thunk(tt, n)
                                        for tt in range(0, 8) for n in range(DOUT // TQ)])
                return front, back

            # per-pair PE deficit budget (us): era0/1 sized to drain their
            # filler within the era; era2 sized to the exp-vs-attention gap
            DPP = [0.60, 0.65, 0.57]

            TRUNC = int(os.environ.get("MHA_TRUNC", "99"))  # debug bisect knob
            NUNITS = int(os.environ.get("MHA_UNITS", "999"))
            for e, units in enumerate(eras):
                if e >= TRUNC:
                    break
                hoff = int(os.environ.get("MHA_HOFF", "0"))
                units = units[hoff:hoff + NUNITS]
                front, b = era_sched(e)
                filler.extend(b)
                state = [0.0, 0.0, 0]  # budget, spent, pairs

                def pace(state=state, dpp=DPP[e], front=front):
                    if os.environ.get("MHA_NOFILL"):
                        return
                    state[0] += dpp
                    state[2] += 1
                    while front and front[0][0] <= state[2] + 3:
                        state[1] += W_PROJ
                        front.pop(0)[1]()
                    while filler and state[1] + filler[0][0] * 0.5 <= state[0]:
                        w_, fn = filler.pop(0)
                        state[1] += w_
                        fn()

                for tj, h in units:
                    attention(h, tj, pace)
                    if (tj, h) == (2, 7):
                        filler.extend(wrap(W_TR, [transp_thunk(tt, fc)
                                      for tt in range(8, 12) for fc in range(FT)]))
                        filler.extend(wrap(W_OP, [outproj_thunk(tt, n)
                                      for tt in range(8, 12) for n in range(DOUT // TQ)]))
                    elif tj == 3 and h in (1, 3, 5):
                        filler.extend(wrap(W_TR, [
                            transp_thunk(tt, h // 2) for tt in range(12, 16)]))
                        if h == 5:
                            filler.extend(wrap(0.64, [
                                outproj_part_thunk(tt, n) for tt in range(12, 16)
                                for n in range(DOUT // TQ)]))
                while front:
                    front.pop(0)[1]()
                drain_filler()

            if TRUNC >= 99:
                for tt in range(12, 16):
                    transp_thunk(tt, FT - 1)()
                    for n in range(DOUT // TQ):
                        outproj_fin_thunk(tt, n)()

    nc.compile()
    return nc


def make_mask(TQ=512, NR=4):
    """mneg[p, jj, r, f] = MASKVAL where f < tk_local (tk_local = 64*jj + p
    + 128*r), i.e. query strictly before key inside diagonal block r."""
    p = np.arange(64)[:, None, None, None]
    jj = np.arange(2)[None, :, None, None]
    r = np.arange(NR)[None, None, :, None]
    f = np.arange(TQ)[None, None, None, :]
    m = np.where(f < 64 * jj + p + 128 * r, np.float32(MASKVAL), np.float32(0.0))
    return m.astype(ml_dtypes.float8_e5m2)


def make_idHL():
    """idHL[p, jj, m] = 1 where m == 64*jj + p (two stacked half-identities)."""
    m = np.zeros((64, 2, 128), np.float32)
    for jj in range(2):
        m[np.arange(64), jj, 64 * jj + np.arange(64)] = 1.0
    return m.astype(ml_dtypes.float8_e5m2)


def make_core_inputs(q_b, k_b, v_b, Wq, bq, Wk, bk, Wv, bv, Wo, fsl, TQ=512):
    """Build the in_map for one core. fsl = feature slice for this core's heads."""
    F = fsl.stop - fsl.start
    FT = F // 128
    NR = TQ // 128
    D = Wq.shape[0]
    s8 = np.float32(1.0 / np.sqrt(np.sqrt(np.float32(D))))  # 1/sqrt(32)

    bf = ml_dtypes.bfloat16
    return {
        "qT": np.ascontiguousarray(q_b.T).astype(bf),
        "kT": np.ascontiguousarray(k_b.T).astype(bf),
        "vT": np.ascontiguousarray(v_b.T).astype(bf),
        "Wq": np.ascontiguousarray(np.asarray(Wq[:, fsl]) * s8).astype(bf),
        "Wk": np.ascontiguousarray(np.asarray(Wk[:, fsl]) * s8).astype(bf),
        "Wv": np.ascontiguousarray(Wv[:, fsl]).astype(bf),
        "Wo": np.ascontiguousarray(Wo[fsl, :]).astype(bf),
        "bq": np.ascontiguousarray((np.asarray(bq[fsl]) * s8).reshape(FT, 128).T),
        "bk": np.ascontiguousarray((np.asarray(bk[fsl]) * s8).reshape(FT, 128).T),
        "bv": np.ascontiguousarray(bv[fsl].reshape(1, F)),
        "mneg": make_mask(TQ, NR),
        "idHL": make_idHL(),
        "idT": np.eye(128, dtype=np.float32).astype(ml_dtypes.bfloat16),
    }


_CACHE = {}


def kernel(q, k, v, Wq, bq, Wk, bk, Wv, bv, Wo, bo, _trace=False):
    B, T, D = q.shape
    H, HD = 16, 64
    n_cores = 8
    gpb = n_cores // B            # head-groups per batch element (2)
    F = D // gpb                  # feature columns per core (512)

    key = (T, D, F)
    if key not in _CACHE:
        _CACHE[key] = build_mha_core(T=T, D=D, F=F, DOUT=D, HD=HD, TQ=512,
                                     num_devices=n_cores)
    nc = _CACHE[key]

    q = np.asarray(q, np.float32)
    k = np.asarray(k, np.float32)
    v = np.asarray(v, np.float32)
    in_maps = []
    for c in range(n_cores):
        b, g = c // gpb, c % gpb
        fsl = slice(g * F, (g + 1) * F)
        in_maps.append(make_core_inputs(
            q[b], k[b], v[b], Wq, bq, Wk, bk, Wv, bv, Wo, fsl))

    res = run_bass_kernel_spmd(nc, in_maps, list(range(n_cores)), trace=_trace)
    out = np.zeros((B, T, D), np.float32)
    for c in range(n_cores):
        out[c // gpb] += np.asarray(res.results[c]["out"], np.float32)
    out += np.asarray(bo, np.float32)
    if _trace:
        kernel.last_exec_time_ns = res.exec_time_ns
    return out


# revision 108
# speedup vs baseline: 1.0144x; 1.0012x over previous
"""Trainium2 Bass kernel for causal multi-head attention.

Problem: B=4, T=2048, D=1024, H=16, HD=64, fp32, causal, scale=1/sqrt(D).

Sharding: 4-way batch x 2-way head-group over 8 cores. Core c=(b,g) computes
heads g*8..g*8+7 for batch b and returns the partial output projection
(contracted over its 512 context columns); the host sums the two partials
per batch element and adds bo.

Per-core dataflow (cost model charges a matmul ap_out_free * cycles_per_row
only -- no charge for contraction or output partitions; fp8 DoubleRow runs
at 0.5 cycles/row):
  - Q/K projections (f32r) write fp8e4 Q8/K8 scaled by 1/sqrt(sqrt(D)) each.
    Scores S^T[tk=128, tq] run as fp8e4 DoubleRow matmuls at 0.5 cyc/col --
    2x under bf16: lhsT = K8 block [64, 2, 128] whose j=1 k-tile is zeroed,
    rhs = Q8 [64, 2, N] whose j=1 plane is arbitrary finite data (the next
    ft plane; a zeroed dummy plane backs the last ft), so the second k-tile
    contributes exactly zero while the instruction still gets DoubleRow rate.
  - Causal mask adds -2048 on diagonal blocks via a DoubleRow fp8e5 matmul
    (lhsT = two half-identities [64,2,128], rhs = host-precomputed mask);
    -2048 = -64 after the folded 1/32 score scale, so exp underflows to 0.
  - exp on the scalar engine over block PAIRS ([128,2,512] psum spanning two
    banks) to amortize the fixed activation access cost; output pt is bf16.
  - PV runs TRANSPOSED: lhsT = pt chunk [tk=128, tq=128] (stationary),
    rhs = V tile [tk=128, 65] bf16 (moving, 65 cols = HD + ones column) so
    each accumulation step charges 65 cols instead of 512, and the softmax
    denominator lands as psum column 64 => per-partition normalize on DVE
    (reciprocal + tensor_scalar), no partition-broadcast DMA bounce.
  - ctx (bf16) is transposed back to [feat, tok] with PE transpose against a
    bf16 identity, then the output projection runs in bf16.
  - Projections / out-proj / transposes are emitted as filler thunks paced
    into the attention stream so the PE never starves while the scalar
    engine works through the exps (the attention inner loop is ACT-heavy);
    tj2+tj3 units are interleaved into one era since tj3 alone is exp-bound.
  - All DRAM traffic is bf16 (inputs/weights host-cast, output partials
    upcast on the host): the cost model serializes every DMA transfer, so
    bytes are the only lever there.
  - Odd heads' S operands are mirrored into partition-0-based tiles by
    sbuf->sbuf DMA: DoubleRow matmuls reading at base partition 64 crash
    the device in the full program (layout-sensitive; probes pass).
"""

import os
import numpy as np
from contextlib import ExitStack

import ml_dtypes
import concourse.bass as bass
import concourse.tile as tile
from concourse import bacc
from concourse import mybir
from concourse.bass_utils import run_bass_kernel_spmd

F32 = mybir.dt.float32
F32R = mybir.dt.float32r
BF16 = mybir.dt.bfloat16
F8E4 = mybir.dt.float8e4
F8E5 = mybir.dt.float8e5
AF = mybir.ActivationFunctionType
OP = mybir.AluOpType
DR = mybir.MatmulPerfMode.DoubleRow

MASKVAL = -2048.0  # -64 after the folded 1/32 score scale


def build_mha_core(T, D, F, DOUT, HD=64, TQ=512, num_devices=1):
    """Build the per-core Bass program.

    T: tokens, D: model dim, F: feature columns owned by this core,
    DOUT: output projection width, HD: head dim, TQ: tq tile width.
    """
    NH = F // HD        # local heads (8)
    DT = D // 128       # contraction tiles for projections (8)
    FT = F // 128       # feature 128-tiles (4)
    FTG = FT // 2       # q8/k8 feature tile groups (2)
    NTOK = T // 128     # token 128-tiles (16)
    NTQ = T // TQ       # tq tiles (4)
    NR = TQ // 128      # 128-blocks per tq tile (4)

    nc = bacc.Bacc(None, target_bir_lowering=False, debug=False, num_devices=num_devices)

    qT = nc.dram_tensor("qT", [D, T], BF16, kind="ExternalInput")
    kTd = nc.dram_tensor("kT", [D, T], BF16, kind="ExternalInput")
    vTd = nc.dram_tensor("vT", [D, T], BF16, kind="ExternalInput")
    Wq = nc.dram_tensor("Wq", [D, F], BF16, kind="ExternalInput")   # pre-scaled
    Wk = nc.dram_tensor("Wk", [D, F], BF16, kind="ExternalInput")   # pre-scaled
    Wv = nc.dram_tensor("Wv", [D, F], BF16, kind="ExternalInput")
    Wo = nc.dram_tensor("Wo", [F, DOUT], BF16, kind="ExternalInput")
    bq = nc.dram_tensor("bq", [128, FT], F32, kind="ExternalInput")  # pre-scaled
    bk = nc.dram_tensor("bk", [128, FT], F32, kind="ExternalInput")
    bv = nc.dram_tensor("bv", [1, F], F32, kind="ExternalInput")
    mneg = nc.dram_tensor("mneg", [64, 2, NR, TQ], F8E5, kind="ExternalInput")
    idHL = nc.dram_tensor("idHL", [64, 2, 128], F8E5, kind="ExternalInput")
    idT = nc.dram_tensor("idT", [128, 128], BF16, kind="ExternalInput")
    out = nc.dram_tensor("out", [T, DOUT], BF16, kind="ExternalOutput")

    with tile.TileContext(nc) as tc:
        with ExitStack() as ctx:
            persist = ctx.enter_context(tc.tile_pool(name="persist", bufs=1))
            Q8_sb = persist.tile([128, FT + 1, T], F8E4)
            K8_sb = persist.tile([128, FT, NTOK, 2, 128], F8E4)
            # Odd heads' operands live at partitions 64..127 of Q8/K8, but
            # DoubleRow matmuls reading at base partition 64 crash the device
            # in the full program (build-layout sensitive; standalone probes
            # pass). Mirror the upper halves into base-0 tiles via sbuf->sbuf
            # DMA and read S operands at base 0 for every head.
            Q8b = persist.tile([64, FT + 1, T], F8E4)
            K8b = persist.tile([64, FT, NTOK, 2, 128], F8E4)
            VA_sb = persist.tile([128, NTOK, NH, HD + 1], BF16)
            CTXN = persist.tile([128, NTOK, F], BF16)       # [tq-part, tok-blk, feat]
            CTXT = persist.tile([128, FT, T], BF16)         # [feat-part, ft, tok]
            Wq_sb = persist.tile([128, DT, F], BF16)
            Wk_sb = persist.tile([128, DT, F], BF16)
            Wv_sb = persist.tile([128, DT, F], BF16)
            Wo_sb = persist.tile([128, FT, DOUT], BF16)
            PART = persist.tile([128, 2 * NR, TQ], F32)  # tail outproj partials
            bq_sb = persist.tile([128, FT], F32)
            bk_sb = persist.tile([128, FT], F32)
            bv_sb = persist.tile([128, F], F32)
            mneg_sb = persist.tile([64, 2, NR, TQ], F8E5)
            idHL_sb = persist.tile([64, 2, 128], F8E5)
            idT_sb = persist.tile([128, 128], BF16)

            # PSUM: pS 2x[128,2,512]f32 (2 banks each) + pPV 2x[128,4,65]
            # (1 bank) + pmix 2x[128,512] (1 bank) = 8 banks exactly.
            pS = ctx.enter_context(tc.tile_pool(name="pS", bufs=2, space="PSUM"))
            pPV = ctx.enter_context(tc.tile_pool(name="pPV", bufs=2, space="PSUM"))
            pmix = ctx.enter_context(tc.tile_pool(name="pmix", bufs=2, space="PSUM"))
            ptp = ctx.enter_context(tc.tile_pool(name="ptp", bufs=4))
            xpool = ctx.enter_context(tc.tile_pool(name="xin", bufs=4))
            rpool = ctx.enter_context(tc.tile_pool(name="rcp", bufs=3))
            opool = ctx.enter_context(tc.tile_pool(name="osb", bufs=3))

            # Zero-plane memsets run on the (otherwise idle) Pool engine and
            # as 4-byte views: as plain DVE fp8 memsets they cost ~13us and
            # head-of-line-block the projection copybacks behind them.
            nc.vector.memset(VA_sb[:, :, :, HD:HD + 1], 1.0)
            nc.gpsimd.memset(K8_sb[:, :, :, 1, :].bitcast(F32), 0.0)
            nc.gpsimd.memset(Q8_sb[:, FT, :].bitcast(F32), 0.0)
            nc.gpsimd.memset(K8b[:, :, :, 1, :].bitcast(F32), 0.0)
            nc.gpsimd.memset(Q8b[:, FT, :].bitcast(F32), 0.0)

            def load_w(dst, wdram):
                wr = wdram[:].rearrange("(dt p) f -> p dt f", p=128)
                nc.sync.dma_start(dst[:], wr[:])

            # ---- filler thunk machinery ----
            filler = []  # (weight_us, thunk) pairs

            def drain_filler():
                while filler:
                    filler.pop(0)[1]()

            xstash = {}

            def prefetch_x(xdram, key, tj):
                def go():
                    t_ = xpool.tile([128, DT, TQ], BF16, tag="xin")
                    src = xdram[:, tj * TQ:(tj + 1) * TQ] \
                        .rearrange("(dt p) t -> p dt t", p=128)
                    nc.sync.dma_start(t_[:], src)
                    xstash[key] = t_
                return go

            def qk_proj_ft(which, tj, ft):
                """Project one 128-feature tile; write fp8 into Q8/K8."""
                Wsb, bsb = (Wq_sb, bq_sb) if which == "q" else (Wk_sb, bk_sb)
                key = (which, tj)

                def go():
                    xt = xstash[key]
                    ps = pmix.tile([128, TQ], F32, tag="pmix")
                    for dt in range(DT):
                        nc.tensor.matmul(
                            ps[:],
                            lhsT=Wsb[:, dt, ft * 128:(ft + 1) * 128],
                            rhs=xt[:, dt, :],
                            start=(dt == 0), stop=(dt == DT - 1))
                    if which == "q":
                        dst = Q8_sb[:, ft, tj * TQ:(tj + 1) * TQ]
                    else:
                        dst = K8_sb[:, ft, tj * NR:(tj + 1) * NR, 0, :]
                    nc.vector.tensor_scalar(
                        dst, ps[:], bsb[:, ft:ft + 1], None, OP.add)
                    # mirror the odd head's half to the base-0 tile
                    if which == "q":
                        nc.sync.dma_start(
                            Q8b[:, ft, tj * TQ:(tj + 1) * TQ],
                            Q8_sb[64:128, ft, tj * TQ:(tj + 1) * TQ])
                    else:
                        nc.sync.dma_start(
                            K8b[:, ft, tj * NR:(tj + 1) * NR, 0, :],
                            K8_sb[64:128, ft, tj * NR:(tj + 1) * NR, 0, :])
                return go

            def v_proj_c(key, tj, c):
                """Project one 128-token block of V (+bias) into VA bf16."""
                def go():
                    xt = xstash[key]
                    tt = tj * NR + c
                    ps = pmix.tile([128, TQ], F32, tag="pmix")
                    psv = ps[:, :F]
                    for dt in range(DT):
                        nc.tensor.matmul(
                            psv,
                            lhsT=xt[:, dt, c * 128:(c + 1) * 128],
                            rhs=Wv_sb[:, dt, :],
                            start=(dt == 0), stop=(dt == DT - 1))
                    nc.vector.tensor_tensor(
                        VA_sb[:, tt, :, 0:HD],
                        psv.rearrange("p (h d) -> p h d", h=NH),
                        bv_sb[:].rearrange("p (h d) -> p h d", h=NH),
                        OP.add)
                return go

            def proj_thunks(which, tj, skip_prefetch=False):
                """Thunk list for projecting input `which` at tile tj."""
                key = (which, tj)
                if which == "v":
                    th = [v_proj_c(key, tj, c) for c in range(NR)]
                else:
                    th = [qk_proj_ft(which, tj, ft) for ft in range(FT)]
                if not skip_prefetch:
                    xdram = {"q": qT, "k": kTd, "v": vTd}[which]
                    th = [prefetch_x(xdram, key, tj)] + th
                return th

            def transp_thunk(tt, fc, act_copy=False):
                """Transpose ctx chunk [tq=128, feat=128] -> CTXT. The tail
                copies run on the scalar engine (idle there) to shorten the
                final DVE chain."""
                def go():
                    pt_ = pmix.tile([128, 128], BF16, tag="pmix")
                    nc.tensor.transpose(
                        pt_[:], CTXN[:, tt, fc * 128:(fc + 1) * 128], idT_sb[:])
                    dst = CTXT[:, fc, tt * 128:(tt + 1) * 128]
                    if act_copy:
                        nc.scalar.copy(dst, pt_[:])
                    else:
                        nc.vector.tensor_copy(dst, pt_[:])
                return go

            ostash = {}

            def outproj_part_thunk(tt, n):
                """ft 0..2 partial for a tail token block (heads 0..5 only)."""
                def go():
                    ps = pmix.tile([128, TQ], F32, tag="pmix")
                    for ft in range(FT - 1):
                        nc.tensor.matmul(
                            ps[:],
                            lhsT=CTXT[:, ft, tt * 128:(tt + 1) * 128],
                            rhs=Wo_sb[:, ft, n * TQ:(n + 1) * TQ],
                            start=(ft == 0), stop=(ft == FT - 2))
                    nc.vector.tensor_copy(PART[:, (tt - 12) * 2 + n, :], ps[:])
                return go

            def outproj_fin_thunk(tt, n):
                """last-ft matmul + partial add fused into the evacuation."""
                def go():
                    ps = pmix.tile([128, TQ], F32, tag="pmix")
                    nc.tensor.matmul(
                        ps[:],
                        lhsT=CTXT[:, FT - 1, tt * 128:(tt + 1) * 128],
                        rhs=Wo_sb[:, FT - 1, n * TQ:(n + 1) * TQ],
                        start=True, stop=True)
                    if n == 0:
                        ot_tile = opool.tile([128, DOUT], BF16, tag="ot")
                        ostash[tt] = ot_tile
                    ot = ostash[tt]
                    nc.vector.tensor_tensor(
                        ot[:, n * TQ:(n + 1) * TQ], ps[:],
                        PART[:, (tt - 12) * 2 + n, :], OP.add)
                    if n == DOUT // TQ - 1:
                        nc.sync.dma_start(
                            out[tt * 128:(tt + 1) * 128, :], ot[:])
                        del ostash[tt]
                return go

            def outproj_thunk(tt, n):
                def go():
                    ps = pmix.tile([128, TQ], F32, tag="pmix")
                    for ft in range(FT):
                        nc.tensor.matmul(
                            ps[:],
                            lhsT=CTXT[:, ft, tt * 128:(tt + 1) * 128],
                            rhs=Wo_sb[:, ft, n * TQ:(n + 1) * TQ],
                            start=(ft == 0), stop=(ft == FT - 1))
                    if n == 0:
                        ot_tile = opool.tile([128, DOUT], BF16, tag="ot")
                        ostash[tt] = ot_tile
                    ot = ostash[tt]
                    nc.vector.tensor_copy(ot[:, n * TQ:(n + 1) * TQ], ps[:])
                    if n == DOUT // TQ - 1:
                        nc.sync.dma_start(
                            out[tt * 128:(tt + 1) * 128, :], ot[:])
                        del ostash[tt]
                return go

            # ---- attention unit ----
            def emit_S_half(pS_t, half, h, tj, i):
                ft = h // 2
                Ksrc, Qsrc = (K8_sb, Q8_sb) if h % 2 == 0 else (K8b, Q8b)
                r = i - NR * tj
                c0 = 128 * r if r > 0 else 0
                # The exp reads the pair tile from the pair's base offset, so
                # the mask matmul must initialize (fully-masked) columns down
                # to the first half's offset on the steeper half.
                c0m = 128 * (r - half) if r - half > 0 else 0
                chunks = []
                n0 = c0
                while n0 < TQ:
                    chunks.append((n0, min(n0 + 256, TQ), "S"))
                    n0 = min(n0 + 256, TQ)
                if r >= 0:
                    # mask-only strip (no S written there) must be its own
                    # chunk: a matmul may not straddle the pending-zero edge
                    if c0m < c0:
                        chunks.append((c0m, c0, "M"))
                    n0 = c0
                    while n0 < TQ:
                        chunks.append((n0, min(n0 + 256, TQ), "M"))
                        n0 = min(n0 + 256, TQ)
                for idx, (a, b, kind) in enumerate(chunks):
                    last = idx == len(chunks) - 1
                    if kind == "S":
                        nc.tensor.matmul(
                            pS_t[:, half, a:b],
                            lhsT=Ksrc[0:64, ft, i, :, :],
                            rhs=Qsrc[0:64, ft:ft + 2, tj * TQ + a:tj * TQ + b],
                            start=(idx == 0), stop=last, perf_mode=DR)
                    else:
                        nc.tensor.matmul(
                            pS_t[:, half, a:b],
                            lhsT=idHL_sb[:],
                            rhs=mneg_sb[:, :, r, a:b],
                            start=False, stop=last, perf_mode=DR)

            def attention(h, tj, pace):
                nblk = NR * (tj + 1)
                pairs = nblk // 2

                def emit_pair(p):
                    t = pS.tile([128, 2, TQ], F32, tag="pS")
                    emit_S_half(t, 0, h, tj, 2 * p)
                    emit_S_half(t, 1, h, tj, 2 * p + 1)
                    return t

                pv_t = pPV.tile([128, NR, HD + 1], F32, tag="pPV")
                pS_cur = emit_pair(0)
                for p in range(pairs):
                    pS_next = emit_pair(p + 1) if p + 1 < pairs else None
                    r0 = 2 * p - NR * tj
                    c0p = 128 * r0 if r0 > 0 else 0
                    pt_t = ptp.tile([128, 2, TQ], BF16, tag="pt")
                    nc.scalar.activation(
                        pt_t[:, :, c0p:], pS_cur[:, :, c0p:], AF.Exp)
                    pace()
                    for half in (0, 1):
                        i = 2 * p + half
                        r = i - NR * tj
                        for q in range(max(r, 0), NR):
                            nc.tensor.matmul(
                                pv_t[:, q, :],
                                lhsT=pt_t[:, half, q * 128:(q + 1) * 128],
                                rhs=VA_sb[:, i, h, :],
                                start=(i == 0 and q == 0),
                                stop=(i == nblk - 1 and q == NR - 1))
                    pS_cur = pS_next
                # normalize: reciprocal of denominators, then scale ctx rows
                rcp = rpool.tile([128, NR], F32, tag="rcp")
                nc.vector.reciprocal(rcp[:], pv_t[:, :, HD])
                for r in range(NR):
                    nc.vector.tensor_scalar(
                        CTXN[:, tj * NR + r, h * HD:(h + 1) * HD],
                        pv_t[:, r, 0:HD],
                        rcp[:, r:r + 1], None, OP.mult)

            # ---- prologue: weights + tj=0 projections, DMA just-in-time.
            # The serial DMA stream is ordered so the first v-proj matmul's
            # exact operands (Wv dt 0-3 + xv dt 0-3, cols 0-255) land first;
            # everything else (second halves, bias) queues behind.
            wvr = Wv[:].rearrange("(dt p) f -> p dt f", p=128)
            qdt = DT // 4
            xv0 = xpool.tile([128, DT, TQ], BF16, tag="xin")
            xv0_src = vTd[:, 0:TQ].rearrange("(dt p) t -> p dt t", p=128)
            for i in range(4):
                a, b_ = i * qdt, (i + 1) * qdt
                nc.sync.dma_start(Wv_sb[:, a:b_, :], wvr[:, a:b_, :])
                nc.sync.dma_start(
                    xv0[:, a:b_, :TQ // 2], xv0_src[:, a:b_, :TQ // 2])
            nc.sync.dma_start(bv_sb[:], bv[:].to_broadcast([128, F]))
            xstash[("v", 0)] = xv0
            vth = proj_thunks("v", 0, True)
            vth[0]()
            vth[1]()
            nc.sync.dma_start(xv0[:, :, TQ // 2:], xv0_src[:, :, TQ // 2:])
            vth[2]()
            vth[3]()
            # q AND k loads issue before the q-proj thunks: xk0 must not sit
            # behind the q-mirror DMAs or the k-projection stalls ~3us
            load_w(Wq_sb, Wq)
            nc.sync.dma_start(bq_sb[:], bq[:])
            prefetch_x(qT, ("q", 0), 0)()
            load_w(Wk_sb, Wk)
            nc.sync.dma_start(bk_sb[:], bk[:])
            prefetch_x(kTd, ("k", 0), 0)()
            nc.sync.dma_start(mneg_sb[:], mneg[:])
            nc.sync.dma_start(idHL_sb[:], idHL[:])
            for th in proj_thunks("q", 0, True):
                th()
            for th in proj_thunks("k", 0, True):
                th()
            # era0's tj1 prefetches issue from the prologue for DMA lead.
            # q1 first: its proj thunks are era0 filler (consumed ~30us)
            # while v1/k1 feed era1's front (~47us).
            prefetch_x(qT, ("q", 1), 1)()
            prefetch_x(vTd, ("v", 1), 1)()
            prefetch_x(kTd, ("k", 1), 1)()
            nc.sync.dma_start(idT_sb[:], idT[:])
            nc.sync.dma_start(
                Wo_sb[:], Wo[:].rearrange("(ft p) n -> p ft n", p=128))

            # ---- filler schedule per tj stretch ----
            # tj0 hosts proj(1); tj1 hosts proj(2); tj2 hosts qproj(3),
            # outproj(0), transp(0); tj3 hosts k/v proj(3) first (needed by
            # its own diagonal), then transp/outproj of tj 1-2. Remaining
            # tail after tj3: transp(3)+outproj(3).
            # ---- eras: tj0 | tj1 | tj2+tj3 interleaved ----
            # tj3 units are ACT(exp)-heavy vs their own PE work; mixing them
            # with tj2 units plus all deferrable PE filler keeps the tensor
            # engine dense while the scalar engine grinds through the exps.
            # `front` thunks are dependency-critical (this era's own K/V
            # projections, popped exactly 1/pair so each lands just before
            # its first consumer); `back` thunks are freely placeable filler
            # paced evenly over the era's remaining pairs.
            eras = [
                [(0, h) for h in range(NH)],
                [(1, h) for h in range(NH)],
                [(2, 0), (2, 1), (2, 2), (3, 0), (2, 3), (2, 4), (3, 1),
                 (2, 5), (2, 6), (3, 2), (2, 7), (3, 3), (3, 4), (3, 5),
                 (3, 6), (3, 7)],
            ]

            # thunk weights = approximate PE microseconds
            W_PROJ, W_OP, W_TR = 1.71, 0.85, 0.053

            def wrap(w_, ths):
                return [(w_, t) for t in ths]

            def era_sched(e):
                front, back = [], []
                if e == 0:
                    back += wrap(W_PROJ, proj_thunks("q", 1, True))
                elif e == 1:
                    # k(1)/v(1) self-feed: h0 needs K8 blocks 4,5 by its
                    # pair-2 S emission and VA 4..7 by pairs 2-3. front =
                    # (deadline_pair, thunk), popped just-in-time.
                    kf1 = [qk_proj_ft("k", 1, ft) for ft in range(FT)]
                    vc1 = [v_proj_c(("v", 1), 1, c) for c in range(NR)]
                    front += [(1, kf1[0]), (2, vc1[0]), (3, vc1[1]), (3, vc1[2]),
                              (4, vc1[3]), (7, kf1[1]), (15, kf1[2]), (23, kf1[3])]
                    back += wrap(0, [prefetch_x(qT, ("q", 2), 2)])
                    back += wrap(W_PROJ, proj_thunks("q", 2, True))
                    back += wrap(0, [prefetch_x(kTd, ("k", 2), 2),
                                     prefetch_x(vTd, ("v", 2), 2),
                                     prefetch_x(qT, ("q", 3), 3),
                                     prefetch_x(kTd, ("k", 3), 3),
                                     prefetch_x(vTd, ("v", 3), 3)])
                else:
                    kf2 = [qk_proj_ft("k", 2, ft) for ft in range(FT)]
                    vc2 = [v_proj_c(("v", 2), 2, c) for c in range(NR)]
                    kf3 = [qk_proj_ft("k", 3, ft) for ft in range(FT)]
                    vc3 = [v_proj_c(("v", 3), 3, c) for c in range(NR)]
                    qf3 = [qk_proj_ft("q", 3, ft) for ft in range(FT)]
                    front += [(1, kf2[0]), (2, vc2[0]), (3, vc2[1]), (4, vc2[2]),
                              (5, vc2[3]), (12, kf2[1]),
                              (13, qf3[0]), (14, qf3[1]), (15, qf3[2]), (16, qf3[3]),
                              (20, kf3[0]), (21, vc3[0]), (22, vc3[1]),
                              (23, vc3[2]), (24, vc3[3]),
                              (30, kf2[2]), (38, kf3[1]), (50, kf2[3]),
                              (76, kf3[2]), (92, kf3[3])]
                    back += wrap(W_TR, [transp_thunk(tt, fc)
                                        for tt in range(0, 8) for fc in range(FT)])
                    back += wrap(W_OP, [outproj_thunk(tt, n)
                                        for tt in range(0, 8) for n in range(DOUT // TQ)])
                return front, back

            # per-pair PE deficit budget (us): era0/1 sized to drain their
            # filler within the era; era2 sized to the exp-vs-attention gap
            DPP = [0.60, 0.65, 0.57]

            TRUNC = int(os.environ.get("MHA_TRUNC", "99"))  # debug bisect knob
            NUNITS = int(os.environ.get("MHA_UNITS", "999"))
            for e, units in enumerate(eras):
                if e >= TRUNC:
                    break
                hoff = int(os.environ.get("MHA_HOFF", "0"))
                units = units[hoff:hoff + NUNITS]
                front, b = era_sched(e)
                filler.extend(b)
                state = [0.0, 0.0, 0]  # budget, spent, pairs

                def pace(state=state, dpp=DPP[e], front=front):
                    if os.environ.get("MHA_NOFILL"):
                        return
                    state[0] += dpp
                    state[2] += 1
                    while front and front[0][0] <= state[2] + 3:
                        state[1] += W_PROJ
                        front.pop(0)[1]()
                    while filler and state[1] + filler[0][0] * 0.5 <= state[0]:
                        w_, fn = filler.pop(0)
                        state[1] += w_
                        fn()

                for tj, h in units:
                    attention(h, tj, pace)
                    if (tj, h) == (2, 7):
                        filler.extend(wrap(W_TR, [transp_thunk(tt, fc)
                                      for tt in range(8, 12) for fc in range(FT)]))
                        filler.extend(wrap(W_OP, [outproj_thunk(tt, n)
                                      for tt in range(8, 12) for n in range(DOUT // TQ)]))
                    elif tj == 3 and h in (1, 3, 5):
                        filler.extend(wrap(W_TR, [
                            transp_thunk(tt, h // 2) for tt in range(12, 16)]))
                        if h == 5:
                            filler.extend(wrap(0.64, [
                                outproj_part_thunk(tt, n) for tt in range(12, 16)
                                for n in range(DOUT // TQ)]))
                while front:
                    front.pop(0)[1]()
                drain_filler()

            if TRUNC >= 99:
                for tt in range(12, 16):
                    transp_thunk(tt, FT - 1, act_copy=True)()
                    for n in range(DOUT // TQ):
                        outproj_fin_thunk(tt, n)()

    nc.compile()
    return nc


def make_mask(TQ=512, NR=4):
    """mneg[p, jj, r, f] = MASKVAL where f < tk_local (tk_local = 64*jj + p
    + 128*r), i.e. query strictly before key inside diagonal block r."""
    p = np.arange(64)[:, None, None, None]
    jj = np.arange(2)[None, :, None, None]
    r = np.arange(NR)[None, None, :, None]
    f = np.arange(TQ)[None, None, None, :]
    m = np.where(f < 64 * jj + p + 128 * r, np.float32(MASKVAL), np.float32(0.0))
    return m.astype(ml_dtypes.float8_e5m2)


def make_idHL():
    """idHL[p, jj, m] = 1 where m == 64*jj + p (two stacked half-identities)."""
    m = np.zeros((64, 2, 128), np.float32)
    for jj in range(2):
        m[np.arange(64), jj, 64 * jj + np.arange(64)] = 1.0
    return m.astype(ml_dtypes.float8_e5m2)


def make_core_inputs(q_b, k_b, v_b, Wq, bq, Wk, bk, Wv, bv, Wo, fsl, TQ=512):
    """Build the in_map for one core. fsl = feature slice for this core's heads."""
    F = fsl.stop - fsl.start
    FT = F // 128
    NR = TQ // 128
    D = Wq.shape[0]
    s8 = np.float32(1.0 / np.sqrt(np.sqrt(np.float32(D))))  # 1/sqrt(32)

    bf = ml_dtypes.bfloat16
    return {
        "qT": np.ascontiguousarray(q_b.T).astype(bf),
        "kT": np.ascontiguousarray(k_b.T).astype(bf),
        "vT": np.ascontiguousarray(v_b.T).astype(bf),
        "Wq": np.ascontiguousarray(np.asarray(Wq[:, fsl]) * s8).astype(bf),
        "Wk": np.ascontiguousarray(np.asarray(Wk[:, fsl]) * s8).astype(bf),
        "Wv": np.ascontiguousarray(Wv[:, fsl]).astype(bf),
        "Wo": np.ascontiguousarray(Wo[fsl, :]).astype(bf),
        "bq": np.ascontiguousarray((np.asarray(bq[fsl]) * s8).reshape(FT, 128).T),
        "bk": np.ascontiguousarray((np.asarray(bk[fsl]) * s8).reshape(FT, 128).T),
        "bv": np.ascontiguousarray(bv[fsl].reshape(1, F)),
        "mneg": make_mask(TQ, NR),
        "idHL": make_idHL(),
        "idT": np.eye(128, dtype=np.float32).astype(ml_dtypes.bfloat16),
    }


_CACHE = {}


def kernel(q, k, v, Wq, bq, Wk, bk, Wv, bv, Wo, bo, _trace=False):
    B, T, D = q.shape
    H, HD = 16, 64
    n_cores = 8
    gpb = n_cores // B            # head-groups per batch element (2)
    F = D // gpb                  # feature columns per core (512)

    key = (T, D, F)
    if key not in _CACHE:
        _CACHE[key] = build_mha_core(T=T, D=D, F=F, DOUT=D, HD=HD, TQ=512,
                                     num_devices=n_cores)
    nc = _CACHE[key]

    q = np.asarray(q, np.float32)
    k = np.asarray(k, np.float32)
    v = np.asarray(v, np.float32)
    in_maps = []
    for c in range(n_cores):
        b, g = c // gpb, c % gpb
        fsl = slice(g * F, (g + 1) * F)
        in_maps.append(make_core_inputs(
            q[b], k[b], v[b], Wq, bq, Wk, bk, Wv, bv, Wo, fsl))

    res = run_bass_kernel_spmd(nc, in_maps, list(range(n_cores)), trace=_trace)
    out = np.zeros((B, T, D), np.float32)
    for c in range(n_cores):
        out[c // gpb] += np.asarray(res.results[c]["out"], np.float32)
    out += np.asarray(bo, np.float32)
    if _trace:
        kernel.last_exec_time_ns = res.exec_time_ns
    return out
